# revision 5
# baseline (speedup 1.0000x reference)
"""DenseGrid multi-LOD bilinear embedding lookup on 8 Trainium2 NeuronCores.

Strategy: data-parallel over points (8-way shard). All 8 LOD grids are
host-merged into ONE table indexed by the cell of a 512x512 grid
(c = floor(coord*511)). Each 984B row holds, for every LOD l, the WxW
corner block g_l[b_y:b_y+W, b_x:b_x+W, :] at base b = round(c*(res_l-1)/511
- 0.5) (fp32 math replicated host/device; zero-padded outside the grid),
with W = [3,3,3,3,3,2,5,7].

Device work per point: ONE 984B indirect-DMA gather + per-LOD separable
"hat" weighted reduction:  out_f = sum_j hat(j-py_rel) sum_k hat(k-px_rel)
B[j,k,f],  hat(t) = relu(1-|t|), px_rel = x*(res-1) - b_x.  The hat weights
implement both candidate-cell selection and the bilinear lerp, with no
per-LOD floors (floors appear only for the table cell index and bases).
"""
import numpy as np
import concourse.bacc as bacc
import concourse.bass as bass
import concourse.mybir as mybir
import concourse.tile as tile
from concourse.bass_utils import run_bass_kernel_spmd

FEAT = 4
NUM_LODS = 8
LODS = [2 ** L for L in range(4, 4 + NUM_LODS)]
N_POINTS = 2_000_000
N_CORES = 8
P = 128
PPP = 2048            # points per partition (per core)
CN = 64               # points per partition per chunk
CHUNKS = PPP // CN
PTS_PER_CORE = P * PPP

IDX_RES = 512
ISC = IDX_RES - 1     # 511
WS = [3, 3, 3, 3, 3, 2, 5, 7]
OFFS = np.cumsum([0] + [w * w * FEAT for w in WS]).tolist()
PAYLOAD = OFFS[-1]    # 492 fp16 = 984 B
MAGIC = 12582912.0    # 1.5 * 2**23
HI5 = 510.99998       # clamp so cell <= 510

F16 = mybir.dt.float16
F32 = mybir.dt.float32
ALU = mybir.AluOpType
ACT = mybir.ActivationFunctionType

_cache = {}


def _build_table(grids):
    T = np.zeros((IDX_RES * IDX_RES, PAYLOAD), dtype=np.float16)
    c = np.arange(IDX_RES, dtype=np.int64)
    M32 = np.float32(MAGIC)
    for l, (res, W) in enumerate(zip(LODS, WS)):
        s = res - 1
        if l == 5:
            b = c.copy()
        else:
            # replicate the device's fp32 base computation bit-exactly
            cr = (c.astype(np.float32) * np.float32(s / ISC)).astype(np.float32)
            u = cr - np.float32(0.5)
            b = ((u + M32) - M32).astype(np.int64)
        j = np.arange(W)
        rows = b[:, None] + j[None, :]            # [512, W], may exceed s
        g = np.asarray(grids[l], dtype=np.float16).reshape(res, res, FEAT)
        gp = np.zeros((res + W, res + W, FEAT), dtype=np.float16)
        gp[:res, :res] = g
        blk = gp[rows[:, None, :, None], rows[None, :, None, :]]
        T[:, OFFS[l]:OFFS[l + 1]] = blk.reshape(IDX_RES * IDX_RES, W * W * FEAT)
    return T


def _build_program():
    nc = bacc.Bacc(None, target_bir_lowering=False)
    with tile.TileContext(nc) as tc:
        with tc.tile_pool(name="dram", bufs=1, space="DRAM") as dram, \
             tc.tile_pool(name="io", bufs=2) as io, \
             tc.tile_pool(name="qp", bufs=2) as qp, \
             tc.tile_pool(name="cc", bufs=1) as ccp, \
             tc.tile_pool(name="w2", bufs=2) as w2, \
             tc.tile_pool(name="w1", bufs=1) as w1:
            x_d = dram.tile([P, PPP * 2], F32, kind="ExternalInput")
            T_d = dram.tile([IDX_RES * IDX_RES, PAYLOAD], F16,
                            kind="ExternalInput", name="table")
            co_d = dram.tile([P, 8 + NUM_LODS], F32, kind="ExternalInput",
                             name="consts")
            out_d = dram.tile([P, PPP * NUM_LODS * FEAT], F32,
                              kind="ExternalOutput")

            co = ccp.tile([P, 8 + NUM_LODS], F32, tag="co", name="co")
            nc.sync.dma_start(out=co[:], in_=co_d[:])
            iota7 = co[:, 0:7]                    # values 0..6
            rvec = co[:, 8:8 + NUM_LODS]          # (res_l-1)/511 per LOD

            for ci in range(CHUNKS):
                xt = io.tile([P, CN * 2], F32, tag="x")
                nc.sync.dma_start(out=xt[:],
                                  in_=x_d[:, ci * CN * 2:(ci + 1) * CN * 2])
                x3 = xt[:].rearrange("p (n two) -> p n two", two=2)
                ot = io.tile([P, CN * NUM_LODS * FEAT], F32, tag="o")
                o3 = ot[:].rearrange("p (n f) -> p n f", f=NUM_LODS * FEAT)

                # --- coords scaled by 511 (scalar engine) ---
                xs5 = w2.tile([P, CN], F32, tag="xs5")
                ys5 = w2.tile([P, CN], F32, tag="ys5")
                nc.scalar.activation(out=xs5[:], in_=x3[:, :, 0],
                                     func=ACT.Copy, scale=float(ISC))
                nc.scalar.activation(out=ys5[:], in_=x3[:, :, 1],
                                     func=ACT.Copy, scale=float(ISC))

                # --- table cell: c = round(min(v, HI5) - 0.5) ---
                def cellf(v, tag):
                    u = w1.tile([P, CN], F32, tag=tag + "u")
                    nc.vector.tensor_scalar(out=u[:], in0=v[:], scalar1=HI5,
                                            scalar2=0.5, op0=ALU.min,
                                            op1=ALU.subtract)
                    t = w2.tile([P, CN], F32, tag=tag)
                    nc.vector.tensor_scalar(out=t[:], in0=u[:], scalar1=MAGIC,
                                            scalar2=MAGIC, op0=ALU.add,
                                            op1=ALU.subtract)
                    return t

                cxf = cellf(xs5, "cxf")
                cyf = cellf(ys5, "cyf")
                idf = w1.tile([P, CN], F32, tag="idf")
                nc.vector.scalar_tensor_tensor(
                    out=idf[:], in0=cyf[:], scalar=float(IDX_RES), in1=cxf[:],
                    op0=ALU.mult, op1=ALU.add)
                idx = w2.tile([P, CN], mybir.dt.int32, tag="idx")
                nc.vector.tensor_copy(out=idx[:], in_=idf[:])

                # --- gather payloads: one 984B row per point ---
                qt = qp.tile([P, CN * PAYLOAD], F16, tag="q")
                for j in range(CN):
                    nc.gpsimd.indirect_dma_start(
                        out=qt[:, j * PAYLOAD:(j + 1) * PAYLOAD],
                        out_offset=None, in_=T_d[:],
                        in_offset=bass.IndirectOffsetOnAxis(
                            ap=idx[:, j:j + 1], axis=0))
                q3 = qt[:].rearrange("p (n v) -> p n v", v=PAYLOAD)

                # --- px_rel/py_rel for all LODs: px = v5*r_l; b = round(
                #     c*r_l - 0.5); rel = px - b.  L5 slice: rel = v5 - c. ---
                def rels(v5, cf, tagp):
                    pr = w2.tile([P, CN * NUM_LODS], F32, tag=tagp)
                    pr3 = pr[:].rearrange("p (n l) -> p n l", l=NUM_LODS)
                    bu = w1.tile([P, CN * NUM_LODS], F32, tag=tagp + "b")
                    bu3 = bu[:].rearrange("p (n l) -> p n l", l=NUM_LODS)
                    rb = rvec.unsqueeze(1).broadcast_to([P, CN, NUM_LODS])
                    nc.vector.tensor_mul(
                        out=pr3,
                        in0=v5[:].unsqueeze(2).broadcast_to([P, CN, NUM_LODS]),
                        in1=rb)
                    nc.vector.tensor_mul(
                        out=bu3,
                        in0=cf[:].unsqueeze(2).broadcast_to([P, CN, NUM_LODS]),
                        in1=rb)
                    nc.vector.tensor_scalar(out=bu[:], in0=bu[:], scalar1=0.5,
                                            scalar2=None, op0=ALU.subtract)
                    nc.vector.tensor_scalar(out=bu[:], in0=bu[:], scalar1=MAGIC,
                                            scalar2=MAGIC, op0=ALU.add,
                                            op1=ALU.subtract)
                    nc.vector.tensor_sub(out=pr3, in0=pr3, in1=bu3)
                    nc.vector.tensor_sub(out=pr3[:, :, 5], in0=v5[:],
                                         in1=cf[:])
                    return pr3

                pxr3 = rels(xs5, cxf, "pxr")
                pyr3 = rels(ys5, cyf, "pyr")

                # --- per LOD: hat weights + separable weighted reduction ---
                for l, (res, W) in enumerate(zip(LODS, WS)):
                    def hats(p3, tag):
                        d = w1.tile([P, CN * W], F32, tag=tag + "d")
                        d3 = d[:].rearrange("p (n w) -> p n w", w=W)
                        nc.vector.tensor_sub(
                            out=d3,
                            in0=p3[:, :, l].unsqueeze(2).broadcast_to(
                                [P, CN, W]),
                            in1=iota7[:, 0:W].unsqueeze(1).broadcast_to(
                                [P, CN, W]))
                        nc.scalar.activation(out=d[:], in_=d[:], func=ACT.Abs)
                        w16 = w2.tile([P, CN * W], F16, tag=tag + "w")
                        nc.scalar.activation(out=w16[:], in_=d[:],
                                             func=ACT.Relu, scale=-1.0,
                                             bias=1.0)
                        return w16[:].rearrange("p (n w) -> p n w", w=W)

                    wx = hats(pxr3, "hx")
                    wy = hats(pyr3, "hy")

                    blk = q3[:, :, OFFS[l]:OFFS[l + 1]].rearrange(
                        "p n (j kf) -> p n j kf", j=W)
                    m1 = w1.tile([P, CN * 7 * 7 * FEAT], F16, tag="m1")
                    m1v = m1[:, 0:CN * W * W * FEAT].rearrange(
                        "p (n j kf) -> p n j kf", j=W, kf=W * FEAT)
                    nc.vector.tensor_mul(
                        out=m1v, in0=blk,
                        in1=wy.unsqueeze(3).broadcast_to([P, CN, W, W * FEAT]))
                    r1 = w1.tile([P, CN * 7 * FEAT], F16, tag="r1")
                    r1v = r1[:, 0:CN * W * FEAT].rearrange(
                        "p (n kf) -> p n kf", kf=W * FEAT)
                    nc.vector.tensor_add(out=r1v, in0=m1v[:, :, 0, :],
                                         in1=m1v[:, :, 1, :])
                    for jj in range(2, W):
                        nc.vector.tensor_add(out=r1v, in0=r1v,
                                             in1=m1v[:, :, jj, :])
                    m2 = w1.tile([P, CN * 7 * FEAT], F16, tag="m2")
                    m2v = m2[:, 0:CN * W * FEAT].rearrange(
                        "p (n k f) -> p n k f", k=W, f=FEAT)
                    nc.vector.tensor_mul(
                        out=m2v,
                        in0=r1v.rearrange("p n (k f) -> p n k f", f=FEAT),
                        in1=wx.unsqueeze(3).broadcast_to([P, CN, W, FEAT]))
                    osl = o3[:, :, l * FEAT:(l + 1) * FEAT]
                    if W == 2:
                        nc.vector.tensor_add(out=osl, in0=m2v[:, :, 0, :],
                                             in1=m2v[:, :, 1, :])
                    else:
                        acc = w1.tile([P, CN * FEAT], F16, tag="acc")
                        ac3 = acc[:].rearrange("p (n f) -> p n f", f=FEAT)
                        nc.vector.tensor_add(out=ac3, in0=m2v[:, :, 0, :],
                                             in1=m2v[:, :, 1, :])
                        for kk in range(2, W - 1):
                            nc.vector.tensor_add(out=ac3, in0=ac3,
                                                 in1=m2v[:, :, kk, :])
                        nc.vector.tensor_add(out=osl, in0=ac3,
                                             in1=m2v[:, :, W - 1, :])

                nc.sync.dma_start(
                    out=out_d[:, ci * CN * NUM_LODS * FEAT:
                              (ci + 1) * CN * NUM_LODS * FEAT],
                    in_=ot[:])
    nc.compile()
    names = {"x": x_d.name, "T": T_d.name, "co": co_d.name, "out": out_d.name}
    return nc, names


def kernel(**inputs):
    x = np.asarray(inputs["x"], dtype=np.float32)
    assert x.shape == (N_POINTS, 2), x.shape
    if "prog" not in _cache:
        _cache["prog"] = _build_program()
    nc, names = _cache["prog"]

    grids = [inputs[f"grid_{i}"] for i in range(NUM_LODS)]
    T = _build_table(grids)

    co = np.zeros((P, 8 + NUM_LODS), dtype=np.float32)
    co[:, 0:7] = np.arange(7, dtype=np.float32)[None, :]
    co[:, 8:8 + NUM_LODS] = np.array(
        [(r - 1) / ISC for r in LODS], dtype=np.float32)[None, :]

    total = N_CORES * PTS_PER_CORE
    x_pad = np.full((total, 2), 0.5, dtype=np.float32)
    x_pad[:N_POINTS] = x
    x_sh = x_pad.reshape(N_CORES, P, PPP, 2).reshape(N_CORES, P, PPP * 2)

    in_maps = [{names["x"]: x_sh[c], names["T"]: T, names["co"]: co}
               for c in range(N_CORES)]

    res = run_bass_kernel_spmd(nc, in_maps, core_ids=list(range(N_CORES)))
    out = np.empty((total, NUM_LODS * FEAT), dtype=np.float32)
    for c in range(N_CORES):
        out[c * PTS_PER_CORE:(c + 1) * PTS_PER_CORE] = np.array(
            res.results[c][names["out"]]).reshape(P * PPP, NUM_LODS * FEAT)
    return out[:N_POINTS]
